# revision 10
# baseline (speedup 1.0000x reference)
"""Trainium2 Bass kernel for nn_AbstractNN_40965398069641.

Computes, given x [8194, 2048] (row0=center, rows 1..8192=eps, row -1=noise),
x_true [32, 2048], W [2048, 2048], b [2048], sparse perturbations
(w_val/w_idx [4096], b_val/b_idx [2048]):

  x_out      = [center@W.T+b ; eps@W.T ; onehot_w ; onehot_b ; noise@|W|.T]
  x_min/max  = x_out[0] -/+ sum(|x_out[1:]|, axis=0)
  x_true_out = x_true@W.T + b

Sharding: eps rows data-parallel over 8 cores (1024 rows each), W.T
replicated; sparse rows by index chunk (512 w / 256 b per core);
center/noise/x_true are computed as k-slice partials (256 contraction
columns per core) and summed in the AllReduce that also reduces the
per-core |x_out| column sums.
"""

import os
import numpy as np

import concourse.bass as bass
import concourse.bacc as bacc
import concourse.mybir as mybir
import concourse.tile as tile
from concourse.bass_utils import run_bass_kernel_spmd

N_CORES = 8
D = 2048                      # d_in == d_out
KT = D // 128                 # 16 k-tiles
EPR = 8192 // N_CORES         # 1024 eps rows per core
MT = EPR // 128               # 8 m-tiles per core
NW = 4096
NB_ = 2048
WPR = NW // N_CORES           # 512 w rows per core -> 4 col-slots
BPR = NB_ // N_CORES          # 256 b rows per core -> 2 col-slots
KS = D // N_CORES             # 256-wide contraction slice per core
KST = KS // 128               # 2 k-tiles in the slice
NBW = 512                     # matmul moving width (one PSUM bank)
NTI = D // NBW                # 4 n-tiles
F32 = mybir.dt.float32
BF16 = mybir.dt.bfloat16
F32R = mybir.dt.float32r
I32 = mybir.dt.int32

_CACHED_NC = None
KLEVEL = int(os.environ.get('KLEVEL', '4'))


def _build():
    nc = bacc.Bacc("TRN2", target_bir_lowering=False, debug=False,
                   num_devices=N_CORES)

    # ---- per-core I/O ----
    d_xT = nc.dram_tensor("xT", [D, EPR], F32, kind="ExternalInput")
    d_Wt = nc.dram_tensor("Wt", [D, D], F32, kind="ExternalInput")
    d_Wsl = nc.dram_tensor("Wsl", [KS, D], F32, kind="ExternalInput")
    d_ctT = nc.dram_tensor("ctT", [KS, 34], F32, kind="ExternalInput")
    d_center = nc.dram_tensor("center", [1, D], F32, kind="ExternalInput")
    d_bias = nc.dram_tensor("bias", [1, D], F32, kind="ExternalInput")
    d_widx = nc.dram_tensor("widx", [128, WPR // 128], I32, kind="ExternalInput")
    d_wval = nc.dram_tensor("wval", [128, WPR // 128], F32, kind="ExternalInput")
    d_bidx = nc.dram_tensor("bidx", [128, BPR // 128], I32, kind="ExternalInput")
    d_bval = nc.dram_tensor("bval", [128, BPR // 128], F32, kind="ExternalInput")

    d_eps = nc.dram_tensor("eps_out", [EPR, D], F32, kind="ExternalOutput")
    d_addw = nc.dram_tensor("addw_out", [WPR, D], F32, kind="ExternalOutput")
    d_addb = nc.dram_tensor("addb_out", [BPR, D], F32, kind="ExternalOutput")
    d_head = nc.dram_tensor("head_out", [33, D], F32, kind="ExternalOutput")
    d_noise = nc.dram_tensor("noise_out", [1, D], F32, kind="ExternalOutput")
    d_minmax = nc.dram_tensor("minmax_out", [2, D], F32, kind="ExternalOutput")

    # collective bounce buffers (DRAM, non-I/O)
    d_ar_in = nc.dram_tensor("ar_in", [33, 3 * D], F32)
    d_ar_out = nc.dram_tensor("ar_out", [33, 3 * D], F32, addr_space="Shared")

    NWT = WPR // 128  # 4
    NBT = BPR // 128  # 2

    with tile.TileContext(nc) as tc:
        with (
            tc.tile_pool(name="wt", bufs=1) as p_wt,
            tc.tile_pool(name="xt", bufs=3) as p_xt,
            tc.tile_pool(name="outsb", bufs=2) as p_out,
            tc.tile_pool(name="abssb", bufs=3) as p_abs,
            tc.tile_pool(name="consts", bufs=1) as p_const,
            tc.tile_pool(name="scratch", bufs=2) as p_scr,
            tc.tile_pool(name="wsl", bufs=1) as p_wsl,
            tc.tile_pool(name="fin", bufs=1) as p_fin,
            tc.tile_pool(name="pmain", bufs=3, space="PSUM") as pp_main,
            tc.tile_pool(name="pabs", bufs=1, space="PSUM") as pp_abs,
        ):
            # ---------- constants / small loads ----------
            iota_f = p_const.tile([128, D], F32, tag="iota")
            nc.gpsimd.iota(iota_f[:], pattern=[[1, D]], base=0,
                           channel_multiplier=0,
                           allow_small_or_imprecise_dtypes=True)

            center_bc = p_const.tile([128, D], F32, tag="center_bc")
            nc.sync.dma_start(center_bc[0:1, :], d_center[:])
            nc.gpsimd.partition_broadcast(center_bc[:], center_bc[0:1, :])

            ones_bf = p_const.tile([128, 1], BF16, tag="ones")
            nc.vector.memset(ones_bf[:], 1.0)

            widx_sb = p_const.tile([128, NWT], I32, tag="widx")
            nc.sync.dma_start(widx_sb[:], d_widx[:])
            wval_sb = p_const.tile([128, NWT], F32, tag="wval")
            nc.sync.dma_start(wval_sb[:], d_wval[:])
            bidx_sb = p_const.tile([128, NBT], I32, tag="bidx")
            nc.sync.dma_start(bidx_sb[:], d_bidx[:])
            bval_sb = p_const.tile([128, NBT], F32, tag="bval")
            nc.sync.dma_start(bval_sb[:], d_bval[:])

            # rows = idx >> 11, cols = idx & 2047, as f32
            wrows_i = p_const.tile([128, NWT], I32, tag="wrows_i")
            nc.vector.tensor_scalar(wrows_i[:], widx_sb[:], 11, None,
                                    op0=mybir.AluOpType.logical_shift_right)
            wcols_i = p_const.tile([128, NWT], I32, tag="wcols_i")
            nc.vector.tensor_scalar(wcols_i[:], widx_sb[:], 2047, None,
                                    op0=mybir.AluOpType.bitwise_and)
            wrows_f = p_const.tile([128, NWT], F32, tag="wrows_f")
            nc.vector.tensor_copy(wrows_f[:], wrows_i[:])
            wcols_f = p_const.tile([128, NWT], F32, tag="wcols_f")
            nc.vector.tensor_copy(wcols_f[:], wcols_i[:])
            bidx_f = p_const.tile([128, NBT], F32, tag="bidx_f")
            nc.vector.tensor_copy(bidx_f[:], bidx_sb[:])

            # ---------- resident W.T (bf16, cast during DMA) ----------
            wt_sb = p_wt.tile([128, KT * D], BF16, tag="wt")
            wt_v = d_Wt[:].rearrange("(kt p) n -> p kt n", p=128)
            wt_sb_v = wt_sb[:].rearrange("p (kt n) -> p kt n", kt=KT)
            # quarter-column DMAs so n-tile 0 compute can start early
            for quar in range(4):
                sl = slice(quar * NBW, (quar + 1) * NBW)
                nc.gpsimd.dma_start(wt_sb_v[:, :, sl], wt_v[:, :, sl])

            # ---------- abs accumulators (live whole kernel) ----------
            pabs = [pp_abs.tile([1, NBW], F32, tag=f"pabs{q}", name=f"pabs{q}")
                    for q in range(NTI)]

            # ---------- main eps loop ----------
            xt_v = d_xT[:].rearrange("(kt p) m -> p kt m", p=128)
            for mi in range(MT):
                xt_t = p_xt.tile([128, KT * 128], BF16, tag="xt")
                nc.gpsimd.dma_start(
                    xt_t[:].rearrange("p (kt m) -> p kt m", kt=KT),
                    xt_v[:, :, mi * 128:(mi + 1) * 128])
                for nh in range(NTI // 2):
                    o = p_out.tile([128, 2 * NBW], F32, tag="outsb")
                    for nj in range(2):
                        ni = nh * 2 + nj
                        ps = pp_main.tile([128, NBW], F32, tag="pmain")
                        for kt in range(KT):
                            nc.tensor.matmul(
                                ps[:],
                                lhsT=xt_t[:, kt * 128:(kt + 1) * 128],
                                rhs=wt_sb[:, kt * D + ni * NBW:
                                          kt * D + ni * NBW + NBW],
                                start=(kt == 0), stop=(kt == KT - 1))
                        nc.vector.tensor_copy(
                            o[:, nj * NBW:(nj + 1) * NBW], ps[:])
                        a = p_abs.tile([128, NBW], BF16, tag="abssb")
                        nc.scalar.activation(a[:], ps[:],
                                             mybir.ActivationFunctionType.Abs)
                        nc.tensor.matmul(pabs[ni][:], lhsT=ones_bf[:], rhs=a[:],
                                         start=(mi == 0), stop=False,
                                         skip_group_check=True)
                    nc.sync.dma_start(
                        d_eps[mi * 128:(mi + 1) * 128,
                              nh * 2 * NBW:(nh + 1) * 2 * NBW], o[:])

            # ---------- add_w one-hot block ----------
            addw_v = d_addw[:].rearrange("(p t) n -> p t n", t=NWT)
            for t in range(NWT if KLEVEL >= 2 else 0):
                wtile = p_scr.tile([128, D], F32, tag="wtile")
                v = p_scr.tile([128, 1], F32, tag=f"v{t}")
                # v[p] = sum_j (iota[j]==col_p) * center[j]  (the gather)
                nc.vector.scalar_tensor_tensor(
                    wtile[:], in0=iota_f[:], scalar=wcols_f[:, t:t + 1],
                    in1=center_bc[:], op0=mybir.AluOpType.is_equal,
                    op1=mybir.AluOpType.mult, accum_out=v[:])
                val = p_scr.tile([128, 1], F32, tag=f"val{t}")
                nc.vector.tensor_tensor(val[:], v[:], wval_sb[:, t:t + 1],
                                        op=mybir.AluOpType.mult)
                nc.vector.tensor_scalar(wtile[:], iota_f[:],
                                        wrows_f[:, t:t + 1], val[:],
                                        op0=mybir.AluOpType.is_equal,
                                        op1=mybir.AluOpType.mult)
                nc.sync.dma_start(addw_v[:, t, :], wtile[:])
                aval = p_scr.tile([128, 1], F32, tag=f"aval{t}")
                nc.scalar.activation(aval[:], val[:],
                                     mybir.ActivationFunctionType.Abs)
                awtile = p_scr.tile([128, D], BF16, tag="awtile")
                nc.vector.tensor_scalar(awtile[:], iota_f[:],
                                        wrows_f[:, t:t + 1], aval[:],
                                        op0=mybir.AluOpType.is_equal,
                                        op1=mybir.AluOpType.mult)
                for q in range(NTI):
                    nc.tensor.matmul(
                        pabs[q][:],
                        lhsT=ones_bf[:],
                        rhs=awtile[:, q * NBW:(q + 1) * NBW],
                        start=False, stop=False, skip_group_check=True)

            # ---------- add_b one-hot block ----------
            addb_v = d_addb[:].rearrange("(p t) n -> p t n", t=NBT)
            for t in range(NBT if KLEVEL >= 2 else 0):
                btile = p_scr.tile([128, D], F32, tag="wtile")
                nc.vector.tensor_scalar(btile[:], iota_f[:],
                                        bidx_f[:, t:t + 1], bval_sb[:, t:t + 1],
                                        op0=mybir.AluOpType.is_equal,
                                        op1=mybir.AluOpType.mult)
                nc.sync.dma_start(addb_v[:, t, :], btile[:])
                abval = p_scr.tile([128, 1], F32, tag=f"abval{t}")
                nc.scalar.activation(abval[:], bval_sb[:, t:t + 1],
                                     mybir.ActivationFunctionType.Abs)
                abtile = p_scr.tile([128, D], BF16, tag="awtile")
                nc.vector.tensor_scalar(abtile[:], iota_f[:],
                                        bidx_f[:, t:t + 1], abval[:],
                                        op0=mybir.AluOpType.is_equal,
                                        op1=mybir.AluOpType.mult)
                last = (t == NBT - 1)
                for q in range(NTI):
                    nc.tensor.matmul(
                        pabs[q][:],
                        lhsT=ones_bf[:],
                        rhs=abtile[:, q * NBW:(q + 1) * NBW],
                        start=False, stop=last, skip_group_check=True)

            # ---------- k-slice partials: x_true / center / noise ----------
            # ctT dram [KS, 34]: col 0 center.T, 1:33 x_true.T, 33 noise.T
            ct_sb = p_const.tile([128, KST * 34], BF16, tag="ct")
            nc.gpsimd.dma_start(
                ct_sb[:].rearrange("p (kt m) -> p kt m", kt=KST),
                d_ctT[:].rearrange("(kt p) m -> p kt m", p=128))
            wsl_raw = [p_wsl.tile([128, D], BF16, tag=f"wslr{k}", name=f"wslr{k}")
                       for k in range(KST)]
            wsl_abs = [p_wsl.tile([128, D], BF16, tag=f"wsla{k}", name=f"wsla{k}")
                       for k in range(KST)]
            wsl_v = d_Wsl[:].rearrange("(kt p) n -> p kt n", p=128)
            for k in range(KST):
                nc.gpsimd.dma_start(wsl_raw[k][:], wsl_v[:, k, :])
                nc.scalar.activation(wsl_abs[k][:], wsl_raw[k][:],
                                     mybir.ActivationFunctionType.Abs)

            # allreduce payload [33, 3*D]:
            # [0:33, 0:D] center+true partials; [0, D:2D] noise partial;
            # [0, 2D:3D] per-core abs column sums. Other rows of the upper
            # columns are never read.
            ar_in = p_fin.tile([33, 3 * D], F32, tag="arr")
            nc.vector.memset(ar_in[0:33, 0:D], 0.0)
            nc.vector.memset(ar_in[0:33, D:3 * D], 0.0)
            for ni in range(NTI):
                nsl = slice(ni * NBW, (ni + 1) * NBW)
                if KLEVEL >= 3:
                    ps_tc = pp_main.tile([33, NBW], F32, tag="pmain")
                    ps_nz = pp_main.tile([1, NBW], F32, tag="pmain")
                    for k in range(KST):
                        nc.tensor.matmul(ps_tc[:],
                                         lhsT=ct_sb[:, k * 34:k * 34 + 33],
                                         rhs=wsl_raw[k][:, nsl],
                                         start=(k == 0), stop=(k == KST - 1))
                        nc.tensor.matmul(ps_nz[:],
                                         lhsT=ct_sb[:, k * 34 + 33:k * 34 + 34],
                                         rhs=wsl_abs[k][:, nsl],
                                         start=(k == 0), stop=(k == KST - 1))
                    nc.vector.tensor_copy(ar_in[0:33, nsl], ps_tc[:])
                    nc.vector.tensor_copy(
                        ar_in[0:1, D + ni * NBW:D + (ni + 1) * NBW], ps_nz[:])
                nc.vector.tensor_copy(
                    ar_in[0:1, 2 * D + ni * NBW:2 * D + (ni + 1) * NBW],
                    pabs[ni][:])

            nc.sync.dma_start(d_ar_in[:], ar_in[:])
            if KLEVEL >= 4:
                nc.gpsimd.collective_compute(
                    "AllReduce", mybir.AluOpType.add,
                    replica_groups=[list(range(N_CORES))],
                    ins=[d_ar_in[:].opt()], outs=[d_ar_out[:].opt()])
            else:
                nc.sync.dma_start(d_ar_out[:], d_ar_in[:])
            ar_out = p_fin.tile([33, 3 * D], F32, tag="arr")
            nc.sync.dma_start(ar_out[:], d_ar_out[:])

            # ---------- finalize (identical on every core) ----------
            bias_bc = p_fin.tile([33, D], F32, tag="bias_bc")
            nc.sync.dma_start(bias_bc[0:1, :], d_bias[:])
            nc.gpsimd.partition_broadcast(bias_bc[:], bias_bc[0:1, :],
                                          channels=33)
            # head rows (center+true) += bias, in place
            nc.vector.tensor_tensor(ar_out[0:33, 0:D], ar_out[0:33, 0:D],
                                    bias_bc[:], op=mybir.AluOpType.add)
            nc.sync.dma_start(d_head[:], ar_out[0:33, 0:D])
            nc.sync.dma_start(d_noise[:], ar_out[0:1, D:2 * D])

            mm = p_fin.tile([1, 2 * D], F32, tag="mm")
            # mm[0, D:2D] (xmax tmp) = |noise| + abs_sums
            nc.scalar.activation(mm[0:1, D:2 * D], ar_out[0:1, D:2 * D],
                                 mybir.ActivationFunctionType.Abs)
            nc.vector.tensor_tensor(mm[0:1, D:2 * D], mm[0:1, D:2 * D],
                                    ar_out[0:1, 2 * D:3 * D],
                                    op=mybir.AluOpType.add)
            # xmin = center - abs_tot ; xmax = center + abs_tot
            nc.vector.tensor_tensor(mm[0:1, 0:D], ar_out[0:1, 0:D],
                                    mm[0:1, D:2 * D],
                                    op=mybir.AluOpType.subtract)
            nc.vector.tensor_tensor(mm[0:1, D:2 * D], ar_out[0:1, 0:D],
                                    mm[0:1, D:2 * D], op=mybir.AluOpType.add)
            nc.sync.dma_start(d_minmax[0:1, :], mm[0:1, 0:D])
            nc.sync.dma_start(d_minmax[1:2, :], mm[0:1, D:2 * D])

    nc.compile()
    return nc


def _get_nc():
    global _CACHED_NC
    if _CACHED_NC is None:
        _CACHED_NC = _build()
    return _CACHED_NC


def make_in_maps(x, x_true, W, b, w_val, b_val, w_idx, b_idx):
    x = np.ascontiguousarray(np.asarray(x, np.float32))
    x_true = np.ascontiguousarray(np.asarray(x_true, np.float32))
    W = np.ascontiguousarray(np.asarray(W, np.float32))
    b = np.asarray(b, np.float32).reshape(1, D)
    w_val = np.asarray(w_val, np.float32)
    b_val = np.asarray(b_val, np.float32)
    w_idx = np.asarray(w_idx).astype(np.int32)
    b_idx = np.asarray(b_idx).astype(np.int32)

    Wt = np.ascontiguousarray(W.T)
    center = np.ascontiguousarray(x[0:1])
    trueT = x_true.T  # [D, 32]
    in_maps = []
    for c in range(N_CORES):
        ks = slice(KS * c, KS * (c + 1))
        ctT = np.empty((KS, 34), np.float32)
        ctT[:, 0] = x[0, ks]
        ctT[:, 1:33] = trueT[ks]
        ctT[:, 33] = x[-1, ks]
        in_maps.append({
            "xT": np.ascontiguousarray(x[1 + EPR * c: 1 + EPR * (c + 1)].T),
            "Wt": Wt,
            "Wsl": np.ascontiguousarray(Wt[ks]),
            "ctT": ctT,
            "center": center,
            "bias": b,
            "widx": np.ascontiguousarray(w_idx[WPR * c: WPR * (c + 1)]
                                         .reshape(128, WPR // 128)),
            "wval": np.ascontiguousarray(w_val[WPR * c: WPR * (c + 1)]
                                         .reshape(128, WPR // 128)),
            "bidx": np.ascontiguousarray(b_idx[BPR * c: BPR * (c + 1)]
                                         .reshape(128, BPR // 128)),
            "bval": np.ascontiguousarray(b_val[BPR * c: BPR * (c + 1)]
                                         .reshape(128, BPR // 128)),
        })
    return in_maps


def assemble(results):
    x_out = np.empty((2 + 8192 + NW + NB_, D), np.float32)
    r0 = results[0]
    x_out[0] = r0["head_out"][0]
    for c in range(N_CORES):
        x_out[1 + EPR * c: 1 + EPR * (c + 1)] = results[c]["eps_out"]
        x_out[8193 + WPR * c: 8193 + WPR * (c + 1)] = results[c]["addw_out"]
        x_out[8193 + NW + BPR * c: 8193 + NW + BPR * (c + 1)] = \
            results[c]["addb_out"]
    x_out[-1] = r0["noise_out"][0]
    x_true_out = r0["head_out"][1:33].copy()
    x_min = r0["minmax_out"][0].copy()
    x_max = r0["minmax_out"][1].copy()
    return x_out, x_min, x_max, x_true_out


def kernel(x, x_true, W, b, w_val, b_val, w_idx, b_idx):
    nc = _get_nc()
    in_maps = make_in_maps(x, x_true, W, b, w_val, b_val, w_idx, b_idx)
    res = run_bass_kernel_spmd(nc, in_maps, list(range(N_CORES)))
    return assemble(res.results)
